# revision 8
# baseline (speedup 1.0000x reference)
"""Trainium2 Bass kernel for nn_AdditiveAttention (B=8, S=4096, D=1024, H=16).

Sharding: pure data-parallel over batch — 8 NeuronCores, one batch element
per core, weights replicated. No collectives.

Per-core layout: everything transposed (d on partitions, s on free).

v4 structure (vs v3): the K GEMM and the replicated K-logit matmuls are
eliminated. The beta logit is a linear functional of x once gq is known:
  lam_h(s) = ck[:, h]^T x_s,   ck = (Wk ∘ wkl-bcast) @ (gq per head)
and the beta pooling is a tiny s-contraction GEMM against an
s-partitioned fp8 copy of X:
  G[h, i]  = sum_s betaexp[s, h] * X[s, i]        (xs8, 16 DR passes)
  kpool    = (Wk^T (G/Z))  via one more tiny DR GEMM (wk8)
  gk       = gq * (kpool + bk)
The bk part of the beta logit is constant per head and cancels in
softmax; bk re-enters only in the gk finalize. The r-part of the output
carries <1% of the output norm, so the whole pooling chain runs in fp8.

  - Q phase: s-quarter outer (unchanged) — q GEMM bf16, q-logits via
    replicated wqlrep matmuls, ACT-exp/DVE-stt pooling, streaming DMA out.
  - During the Q tail: m8 (head-masked gq) and the CT GEMM accumulate as
    per-tile gq finalizes; ck is replicated 16->128 columns via one
    matmul per d-tile against a 0/1 `sel` matrix.
  - Chain: lam chunks (fp8 DR from xt8) -> ACT exp (no max subtract;
    logits are +-0.03) -> PE transposes -> bexpT8 -> G -> normalize ->
    WGT -> gk, pipelined; then wcomb build + V phase as in v3.
  - V phase: rt = (Wv diag(gk) Wr)^T x as a streaming fp8 DR GEMM.
  - Outputs bf16 (q-residual and rt summed on host in f32).
"""

import sys
import types

import numpy as np
import ml_dtypes

from contextlib import ExitStack

import concourse.bass as bass
import concourse.tile as tile
from concourse import bacc, mybir
from concourse.bass_utils import run_bass_kernel_spmd

B, S, D, H, HD = 8, 4096, 1024, 16, 64
P = 128          # partitions
T = D // P       # 8 d-tiles
NC_ = 512        # psum bank free size
NP = 1024        # paired op width
NG = S // NP     # 4 quarter/pair groups
NS = S // P      # 32 s-tiles
N_CORES = 8
BF16 = mybir.dt.bfloat16
FP8 = mybir.dt.float8e4
F32 = mybir.dt.float32
W8SCALE = 64.0   # host scales Wk/WvT by this into e4m3 normal range
GK8 = 256.0      # device scale for gk -> fp8 operand
WCE = 2.0 ** -4  # psWC -> wcomb8 evacuation scale
VDE = 2.0 ** -18 # V-phase psum -> output scale (undoes 64*64*256*WCE)
ATS = 256.0      # host scale for AT8 = (Wk o wkl)^T
MS = 64.0        # device scale gq -> m8
CKS = 1024.0     # ck8 scale; ct16 evac = CKS/(ATS*MS)
EXS = 2.0 ** -13 # lam-psum -> logit scale = (1/8)/CKS
BES = 8.0        # betaexp -> fp8 scale
GS = 128.0       # Gn -> fp8 scale
WGE = 2.0 ** -13 # WGT psum -> true kpool = 1/(GS*W8SCALE)
BF = ml_dtypes.bfloat16
F8 = ml_dtypes.float8_e4m3

_CACHE = {}


def _build():
    nc = bacc.Bacc(
        "TRN2", target_bir_lowering=False, debug=False, num_devices=N_CORES
    )
    xt_ext = nc.declare_dram_parameter("xt", [D, S], BF16, isOutput=False)
    xs_ext = nc.declare_dram_parameter("xs8", [S, D], FP8, isOutput=False)
    wq_ext = nc.declare_dram_parameter("wq", [D, D], BF16, isOutput=False)
    wk_ext = nc.declare_dram_parameter("wk", [D, D], FP8, isOutput=False)
    at_ext = nc.declare_dram_parameter("at8", [D, D], FP8, isOutput=False)
    wvt_ext = nc.declare_dram_parameter("wvt", [D, D], FP8, isOutput=False)
    bq_ext = nc.declare_dram_parameter("bq", [P, T], F32, isOutput=False)
    bk_ext = nc.declare_dram_parameter("bk", [P, T], F32, isOutput=False)
    bv_ext = nc.declare_dram_parameter("bv", [P, T], F32, isOutput=False)
    wql_ext = nc.declare_dram_parameter("wqlrep", [P, P], BF16, isOutput=False)
    sel_ext = nc.declare_dram_parameter("sel", [P, P], BF16, isOutput=False)
    wrr_ext = nc.declare_dram_parameter("wrr", [P, P], BF16, isOutput=False)
    wrr8_ext = nc.declare_dram_parameter("wrr8", [P, P], FP8, isOutput=False)
    br_ext = nc.declare_dram_parameter("br", [P, 1], F32, isOutput=False)
    out_ext = nc.declare_dram_parameter("out", [D, S], BF16, isOutput=True)
    out2_ext = nc.declare_dram_parameter("out2", [D, S], BF16, isOutput=True)

    AX = mybir.AxisListType.X
    ALU = mybir.AluOpType
    AF = mybir.ActivationFunctionType

    with tile.TileContext(nc) as tc, ExitStack() as ctx:
        singles = ctx.enter_context(tc.tile_pool(name="singles", bufs=1))
        psum = ctx.enter_context(tc.tile_pool(name="psum", bufs=2, space="PSUM"))
        pslg = ctx.enter_context(tc.tile_pool(name="pslg", bufs=2, space="PSUM"))
        gpool = ctx.enter_context(tc.tile_pool(name="gpool", bufs=1, space="PSUM"))
        ch_pool = ctx.enter_context(tc.tile_pool(name="chpool", bufs=5))
        e_pool = ctx.enter_context(tc.tile_pool(name="epool", bufs=4))
        eff_pool = ctx.enter_context(tc.tile_pool(name="eff", bufs=2))
        small_pool = ctx.enter_context(tc.tile_pool(name="small", bufs=4))

        # ---- resident tiles ----
        xt_sb = singles.tile([P, T, S], BF16, name="xt", tag="xt")
        xt8_sb = singles.tile([P, T, S], FP8, name="xt8", tag="xt8")
        xs8_sb = singles.tile([P, NS, D], FP8, name="xs8", tag="xs8")
        wq_sb = singles.tile([P, T, D], BF16, name="wq", tag="wq")
        wk_sb = singles.tile([P, T, D], FP8, name="wk", tag="wk")
        at8_sb = singles.tile([P, T, D], FP8, name="at8", tag="at8")
        wvt_sb = singles.tile([P, T, D], FP8, name="wvt", tag="wvt")
        wcomb_sb = singles.tile([P, T, D], FP8, name="wcomb", tag="wcomb")
        wqlrep = singles.tile([P, P], BF16, name="wqlrep", tag="wqlrep")
        sel_sb = singles.tile([P, P], BF16, name="sel", tag="sel")
        wrr = singles.tile([P, P], BF16, name="wrr", tag="wrr")
        wrr8 = singles.tile([P, P], FP8, name="wrr8", tag="wrr8")
        bq_sb = singles.tile([P, T], F32, name="bq", tag="bq")
        bk_sb = singles.tile([P, T], F32, name="bk", tag="bk")
        bv_sb = singles.tile([P, T], F32, name="bv", tag="bv")
        bvg_sb = singles.tile([P, T], BF16, name="bvg", tag="bvg")
        br_sb = singles.tile([P, 1], F32, name="br", tag="br")
        bias2_sb = singles.tile([P, T], F32, name="bias2", tag="bias2")
        zq_sb = singles.tile([P, T, 2 * NG], F32, name="zq", tag="zq")
        gqp_sb = singles.tile([P, T, NG], F32, name="gqp", tag="gqp")
        gq_all = singles.tile([P, T], F32, name="gq", tag="gq")
        gk_all = singles.tile([P, T], F32, name="gk", tag="gk")
        gk8c = singles.tile([P, 1], F32, name="gk8c", tag="gk8c")
        m8_sb = singles.tile([P, T, H], FP8, name="m8", tag="m8")
        ckrep8 = singles.tile([P, T, P], FP8, name="ckrep8", tag="ckrep8")
        ct16 = singles.tile([P, D], BF16, name="ct16", tag="ct16")
        gn16 = singles.tile([P, D], BF16, name="gn16", tag="gn16")
        wgt16 = singles.tile([P, D], BF16, name="wgt16", tag="wgt16")
        bexpT8 = singles.tile([P, NS, H], FP8, name="bexpT8", tag="bexpT8")
        g8_sb = singles.tile([P, T, H], FP8, name="g8", tag="g8")
        bexp_ring = singles.tile([P, 2, NC_], BF16, name="bexpr", tag="bexpr")
        zkp_sb = singles.tile([P, T], F32, name="zkp", tag="zkp")
        kp_sb = singles.tile([P, 1], F32, name="kp", tag="kp")
        scratch = singles.tile([P, NC_], BF16, name="scr", tag="scr")

        # ---- DMA issue ----
        # sync queue: small weights, then wq in t-major blocks (each block
        # unlocks one output tile's GEMMs — block 0 lands ~5us).
        nc.sync.dma_start(wqlrep[:], wql_ext.ap())
        nc.sync.dma_start(sel_sb[:], sel_ext.ap())
        nc.sync.dma_start(wrr[:], wrr_ext.ap())
        nc.sync.dma_start(wrr8[:], wrr8_ext.ap())
        nc.sync.dma_start(bq_sb[:], bq_ext.ap())
        nc.sync.dma_start(bk_sb[:], bk_ext.ap())
        nc.sync.dma_start(bv_sb[:], bv_ext.ap())
        nc.sync.dma_start(br_sb[:], br_ext.ap())
        nc.sync.dma_start(wq_sb[:, 0, :], wq_ext.ap()[0:P, :])
        # quarter 0 is split across both queues so the first GEMMs unblock
        # as early as possible; wq t-blocks 1..7 follow on sync.
        for k in range(0, T, 2):
            nc.sync.dma_start(
                xt_sb[:, k, 0:NP], xt_ext.ap()[k * P : (k + 1) * P, 0:NP]
            )
        for k in range(1, T, 2):
            nc.gpsimd.dma_start(
                xt_sb[:, k, 0:NP], xt_ext.ap()[k * P : (k + 1) * P, 0:NP]
            )
        for t in range(1, T):
            nc.sync.dma_start(wq_sb[:, t, :], wq_ext.ap()[t * P : (t + 1) * P, :])
        # gpsimd queue: remaining xt quarters (landing in consumption
        # order), then the chain weights. xt8 is derived on-device.
        for g in range(1, NG):
            sl = slice(g * NP, (g + 1) * NP)
            for k in range(T):
                nc.gpsimd.dma_start(
                    xt_sb[:, k, sl], xt_ext.ap()[k * P : (k + 1) * P, sl]
                )
        for k in range(T):
            rsl = slice(k * P, (k + 1) * P)
            nc.gpsimd.dma_start(at8_sb[:, k, :], at_ext.ap()[rsl, :])
        for si in range(NS):
            nc.gpsimd.dma_start(
                xs8_sb[:, si, :], xs_ext.ap()[si * P : (si + 1) * P, :]
            )
        for k in range(T):
            rsl = slice(k * P, (k + 1) * P)
            nc.gpsimd.dma_start(wk_sb[:, k, :], wk_ext.ap()[rsl, :])
        for k in range(T):
            rsl = slice(k * P, (k + 1) * P)
            nc.gpsimd.dma_start(wvt_sb[:, k, :], wvt_ext.ap()[rsl, :])

        # ---- warm-up during the DMA window: HAM un-throttle + exp table ----
        nc.vector.memset(scratch[:], 0.0)
        nc.vector.memset(m8_sb[:], 0.0)
        for _ in range(9):
            warm_ps = pslg.tile([P, NC_], F32, name="plg", tag="plg")
            nc.tensor.matmul(warm_ps[:], scratch[:, 0:P], scratch[:],
                             start=True, stop=True)
            nc.tensor.matmul(warm_ps[:], scratch[:, 0:P], scratch[:],
                             start=True, stop=True)
        nc.scalar.activation(bexp_ring[:, 0, :], warm_ps[:], AF.Exp,
                             bias=0.0, scale=1.0)
        for _ in range(10):
            warm_ps = pslg.tile([P, NC_], F32, name="plg", tag="plg")
            nc.tensor.matmul(
                warm_ps[:], wq_sb[:, 0, 0:P],
                wq_sb[:, 0, 0:NC_], start=True, stop=True,
            )
            nc.tensor.matmul(
                warm_ps[:], wq_sb[:, 0, 0:P],
                wq_sb[:, 0, 0:NC_], start=True, stop=True,
            )

        # ================= Q phase: s-quarter outer =================
        pend = None  # deferred logit matmul for the previous pair
        ct_ps = [None]  # CT accumulator (gpool), allocated at first pass

        def emit_logit_q(t, g, qtc):
            plga = pslg.tile([P, NC_], F32, name="plg", tag="plg")
            nc.tensor.matmul(plga[:], wqlrep[:], qtc[:, 0:NC_],
                             start=True, stop=True)
            plgb = pslg.tile([P, NC_], F32, name="plg", tag="plg")
            nc.tensor.matmul(plgb[:], wqlrep[:], qtc[:, NC_:NP],
                             start=True, stop=True)
            ec = e_pool.tile([P, NP], BF16, name="ec", tag="ec")
            nc.scalar.activation(
                ec[:, 0:NC_], plga[:], AF.Exp, bias=0.0, scale=1.0,
                accum_out=zq_sb[:, t, 2 * g : 2 * g + 1],
            )
            nc.scalar.activation(
                ec[:, NC_:NP], plgb[:], AF.Exp, bias=0.0, scale=1.0,
                accum_out=zq_sb[:, t, 2 * g + 1 : 2 * g + 2],
            )
            nc.vector.scalar_tensor_tensor(
                ec[:], ec[:], 1.0, qtc[:], ALU.mult, ALU.mult,
                accum_out=gqp_sb[:, t, g : g + 1],
            )

        def finalize_q(t):
            ztot = small_pool.tile([P, 1], F32, name="ztot", tag="ztot")
            nc.vector.reduce_sum(ztot, zq_sb[:, t, :], axis=AX)
            recip = small_pool.tile([P, 1], F32, name="recip", tag="recip")
            nc.vector.reciprocal(recip, ztot)
            graw = small_pool.tile([P, 1], F32, name="graw", tag="graw")
            nc.vector.reduce_sum(graw, gqp_sb[:, t, :], axis=AX)
            nc.vector.tensor_mul(gq_all[:, t : t + 1], graw, recip)
            # head-masked scaled gq for the CT GEMM
            nc.vector.tensor_scalar_mul(
                m8_sb[0:HD, t, 2 * t : 2 * t + 1], gq_all[0:HD, t : t + 1], MS
            )
            nc.vector.tensor_scalar_mul(
                m8_sb[HD:P, t, 2 * t + 1 : 2 * t + 2], gq_all[HD:P, t : t + 1], MS
            )

        def emit_ct_pass(p):
            # CT[h, i] = sum_j m8[j, h] * AT8[j, i], DR pair p
            if ct_ps[0] is None:
                ct_ps[0] = gpool.tile([P, NP], F32, name="gacc", tag="gacc")
            for h in range(2):
                hsl = slice(h * NC_, (h + 1) * NC_)
                nc.tensor.matmul(
                    ct_ps[0][0:H, hsl], m8_sb[:, 2 * p : 2 * p + 2, :],
                    at8_sb[:, 2 * p : 2 * p + 2, hsl],
                    start=(p == 0), stop=(p == T // 2 - 1),
                    perf_mode=mybir.MatmulPerfMode.DoubleRow,
                )

        for g in range(NG):
            sl = slice(g * NP, (g + 1) * NP)
            for t in range(T):
                pch = psum.tile([P, NP], F32, name="pch", tag="pch")
                for h in range(2):
                    hsl = slice(g * NP + h * NC_, g * NP + (h + 1) * NC_)
                    psl = slice(h * NC_, (h + 1) * NC_)
                    for k in range(T):
                        nc.tensor.matmul(
                            pch[:, psl],
                            wq_sb[:, t, k * P : (k + 1) * P],
                            xt_sb[:, k, hsl],
                            start=(k == 0),
                            stop=(k == T - 1),
                        )
                qtc = ch_pool.tile([P, NP], BF16, name="qtc", tag="qtc")
                nc.scalar.activation(
                    qtc[:], pch[:], AF.Identity, bias=bq_sb[:, t : t + 1],
                    scale=1.0,
                )
                nc.sync.dma_start(out_ext.ap()[t * P : (t + 1) * P, sl], qtc[:])
                if pend is not None:
                    emit_logit_q(*pend)
                pend = (t, g, qtc)
                # derive the fp8 copy of xt on spare DVE capacity. One
                # k-tile per iteration through quarter 2; quarter 3's
                # conversions are front-loaded so the phase tail keeps DVE
                # free for the finalizers and the chain rampup.
                if g < 3:
                    c = g * T + t
                    nc.vector.tensor_copy(
                        xt8_sb[:, c % T, (c // T) * NP : (c // T + 1) * NP],
                        xt_sb[:, c % T, (c // T) * NP : (c // T + 1) * NP],
                    )
                elif t < 4:
                    for c in (24 + 2 * t, 25 + 2 * t):
                        nc.vector.tensor_copy(
                            xt8_sb[:, c % T, (c // T) * NP : (c // T + 1) * NP],
                            xt_sb[:, c % T, (c // T) * NP : (c // T + 1) * NP],
                        )
                if g == NG - 1 and t > 0:
                    finalize_q(t - 1)
                    if t % 2 == 0:
                        emit_ct_pass(t // 2 - 1)
        emit_logit_q(*pend)
        finalize_q(T - 1)
        emit_ct_pass(T // 2 - 1)

        # ================= beta chain =================
        # ct16 = CKS * ck^T  (CT raw is ATS*MS*ck^T)
        nc.scalar.activation(ct16[0:H, :], ct_ps[0][0:H, :], AF.Identity,
                             bias=0.0, scale=CKS / (ATS * MS))
        # replicate ck 16->128 columns, per d-tile: one matmul vs sel
        for t in range(T):
            rep = pslg.tile([P, NC_], F32, name="plg", tag="plg")
            nc.tensor.matmul(rep[:, 0:P], ct16[0:H, t * P : (t + 1) * P],
                             sel_sb[0:H, :], start=True, stop=True)
            nc.vector.tensor_copy(ckrep8[:, t, :], rep[:, 0:P])

        # lam chunks -> exp -> transposes -> bexpT8; G accumulates in gpool
        g_ps = gpool.tile([P, NP], F32, name="gacc", tag="gacc")

        def emit_lam_chunk(c):
            sl = slice(c * NC_, (c + 1) * NC_)
            lam = pslg.tile([P, NC_], F32, name="plg", tag="plg")
            for k in range(0, T, 2):
                nc.tensor.matmul(
                    lam[:],
                    ckrep8[:, k : k + 2, :],
                    xt8_sb[:, k : k + 2, sl],
                    start=(k == 0), stop=(k == T - 2),
                    perf_mode=mybir.MatmulPerfMode.DoubleRow,
                )
            er = bexp_ring[:, c % 2, :]
            nc.scalar.activation(
                er, lam[:], AF.Exp, bias=0.0, scale=EXS,
                accum_out=zkp_sb[:, c : c + 1],
            )

        def emit_bexp_tp(c):
            for j in range(4):
                si = 4 * c + j
                tp = psum.tile([P, NP], F32, name="pch", tag="pch")
                nc.tensor.matmul(
                    tp[:, 0:H], bexp_ring[0:H, c % 2, j * P : (j + 1) * P],
                    sel_sb[0:H, 0:H], start=True, stop=True,
                )
                nc.vector.tensor_scalar_mul(bexpT8[:, si, :], tp[:, 0:H], BES)

        for c in range(T):
            emit_lam_chunk(c)
            if c >= 1:
                emit_bexp_tp(c - 1)
        emit_bexp_tp(T - 1)
        for si in range(0, NS, 2):
            for h in range(2):
                hsl = slice(h * NC_, (h + 1) * NC_)
                nc.tensor.matmul(
                    g_ps[0:H, hsl], bexpT8[:, si : si + 2, :],
                    xs8_sb[:, si : si + 2, hsl],
                    start=(si == 0), stop=(si == NS - 2),
                    perf_mode=mybir.MatmulPerfMode.DoubleRow,
                )
        # normalize: gn16 = G / (BES * Z)
        zka = small_pool.tile([P, 1], F32, name="ztot", tag="ztot")
        nc.vector.reduce_sum(zka, zkp_sb[:], axis=AX)
        nc.vector.tensor_scalar_mul(zka, zka, BES)
        rz = small_pool.tile([P, 1], F32, name="recip", tag="recip")
        nc.vector.reciprocal(rz, zka)
        nc.vector.tensor_scalar_mul(gn16[0:H, :], g_ps[0:H, :], rz[0:H, :])
        for t in range(T):
            tp = pslg.tile([P, NC_], F32, name="plg", tag="plg")
            nc.tensor.matmul(tp[:, 0:H], gn16[0:H, t * P : (t + 1) * P],
                             sel_sb[0:H, 0:H], start=True, stop=True)
            nc.vector.tensor_scalar_mul(g8_sb[:, t, :], tp[:, 0:H], GS)
        wgt_ps = gpool.tile([P, NP], F32, name="gacc", tag="gacc")
        for k in range(0, T, 2):
            for h in range(2):
                hsl = slice(h * NC_, (h + 1) * NC_)
                nc.tensor.matmul(
                    wgt_ps[0:H, hsl], g8_sb[:, k : k + 2, :],
                    wk_sb[:, k : k + 2, hsl],
                    start=(k == 0), stop=(k == T - 2),
                    perf_mode=mybir.MatmulPerfMode.DoubleRow,
                )
        nc.scalar.activation(wgt16[0:H, :], wgt_ps[0:H, :], AF.Identity,
                             bias=0.0, scale=WGE)

        def finalize_gk(t):
            tp = pslg.tile([P, NC_], F32, name="plg", tag="plg")
            nc.tensor.matmul(tp[:, 0:H], wgt16[0:H, t * P : (t + 1) * P],
                             sel_sb[0:H, 0:H], start=True, stop=True)
            nc.vector.tensor_copy(kp_sb[0:HD, :], tp[0:HD, 2 * t : 2 * t + 1])
            nc.vector.tensor_copy(kp_sb[HD:P, :], tp[HD:P, 2 * t + 1 : 2 * t + 2])
            nc.vector.scalar_tensor_tensor(
                gk_all[:, t : t + 1], kp_sb[:], bk_sb[:, t : t + 1],
                gq_all[:, t : t + 1], ALU.add, ALU.mult,
            )

        def build_wcomb(j):
            """wcomb8[:, :, j-cols] = 2^18 * Wv @ diag(gk_j) @ Wr, block j."""
            nc.vector.tensor_scalar_mul(gk8c[:], gk_all[:, j : j + 1], GK8)
            gkwr = eff_pool.tile([P, P], FP8, name="gkwr", tag="gkwr")
            nc.vector.tensor_scalar_mul(gkwr[:], wrr8[:], gk8c[:, 0:1])
            pw = gpool.tile([P, NP], F32, name="gacc", tag="gacc")
            for tt in range(T):
                nc.tensor.matmul(
                    pw[:, tt * P : (tt + 1) * P],
                    wvt_sb[:, j, tt * P : (tt + 1) * P], gkwr[:],
                    start=True, stop=True,
                )
            nc.scalar.activation(
                wcomb_sb[:, :, j * P : (j + 1) * P], pw[:],
                AF.Identity, bias=0.0, scale=WCE,
            )
            # bias2_j = Wr^T (bv*gk)_j + br    (true scale)
            nc.vector.tensor_mul(
                bvg_sb[:, j : j + 1], bv_sb[:, j : j + 1], gk_all[:, j : j + 1]
            )
            pb = pslg.tile([P, NC_], F32, name="plg", tag="plg")
            nc.tensor.matmul(
                pb[:, 0:1], wrr[:], bvg_sb[:, j : j + 1], start=True, stop=True
            )
            nc.scalar.activation(
                bias2_sb[:, j : j + 1], pb[:, 0:1], AF.Identity,
                bias=br_sb[:, 0:1], scale=1.0,
            )

        def emit_v_pair(j, g):
            """One V-phase GEMM pair for tile j (gate folded into wcomb)."""
            sl = slice(g * NP, (g + 1) * NP)
            pv = psum.tile([P, NP], F32, name="pch", tag="pch")
            for h in range(2):
                hsl = slice(g * NP + h * NC_, g * NP + (h + 1) * NC_)
                psl = slice(h * NC_, (h + 1) * NC_)
                for k in range(0, T, 2):
                    nc.tensor.matmul(
                        pv[:, psl],
                        wcomb_sb[:, k : k + 2, j * P : (j + 1) * P],
                        xt8_sb[:, k : k + 2, hsl],
                        start=(k == 0),
                        stop=(k == T - 2),
                        perf_mode=mybir.MatmulPerfMode.DoubleRow,
                    )
            stg = ch_pool.tile([P, NP], BF16, name="qtc", tag="qtc")
            nc.scalar.activation(
                stg[:], pv[:], AF.Identity,
                bias=bias2_sb[:, j : j + 1], scale=VDE,
            )
            nc.sync.dma_start(out2_ext.ap()[j * P : (j + 1) * P, sl], stg[:])

        # ---- gk finalize + wcomb + V phase, lag-1 interleaved ----
        for t in range(T):
            finalize_gk(t)
            build_wcomb(t)
            if t >= 1:
                for g in range(NG):
                    emit_v_pair(t - 1, g)
        for g in range(NG):
            emit_v_pair(T - 1, g)

    nc.compile()
    return nc


def _prep_shared(inputs):
    """Host-side prep of the replicated (weight) arrays."""
    sc = 0.125  # 1/sqrt(HD)

    def rep_logit(w, scale):
        m = np.zeros((P, P), dtype=np.float32)
        ws = w.astype(np.float32) * scale
        m[:HD, :HD] = ws[:, None]
        m[HD:, HD:] = ws[:, None]
        return m.astype(BF)

    def bias_pp(b):
        return np.ascontiguousarray(b.astype(np.float32).reshape(T, P).T)

    wrrf = np.zeros((P, P), dtype=np.float32)
    wr = inputs["Wr"].astype(np.float32)
    wrrf[:HD, :HD] = wr
    wrrf[HD:, HD:] = wr

    selm = np.zeros((P, P), dtype=np.float32)
    for h in range(H):
        selm[h, h::H] = 1.0

    wq_tmaj = (
        inputs["Wq"].astype(np.float32)
        .reshape(T, P, T, P).transpose(2, 1, 0, 3).reshape(D, D)
    )
    wkf = inputs["Wk"].astype(np.float32)
    wklcol = np.tile(inputs["wkl"].astype(np.float32), H)  # [D] per-j wkl
    at = (wkf * wklcol[None, :]).T  # AT[j, i] = Wk[i, j] * wkl_j
    return {
        "wq": np.ascontiguousarray(wq_tmaj.astype(BF)),
        "wk": np.ascontiguousarray((wkf * W8SCALE).astype(F8)),
        "at8": np.ascontiguousarray((at * ATS).astype(F8)),
        "wvt": np.ascontiguousarray(
            (inputs["Wv"].astype(np.float32).T * W8SCALE).astype(F8)
        ),
        "bq": bias_pp(inputs["bq"]),
        "bk": bias_pp(inputs["bk"]),
        "bv": bias_pp(inputs["bv"]),
        "wqlrep": rep_logit(inputs["wql"], sc),
        "sel": selm.astype(BF),
        "wrr": wrrf.astype(BF),
        "wrr8": (wrrf * W8SCALE).astype(F8),
        "br": np.ascontiguousarray(
            np.tile(inputs["br"].astype(np.float32), 2).reshape(P, 1)
        ),
    }


def _get_nc():
    if "nc" not in _CACHE:
        _CACHE["nc"] = _build()
    return _CACHE["nc"]


def _run(inputs, trace=False):
    nc = _get_nc()
    shared = _prep_shared(inputs)
    X = inputs["X"]
    in_maps = []
    for b in range(N_CORES):
        m = dict(shared)
        m["xt"] = np.ascontiguousarray(X[b].T).astype(BF)
        m["xs8"] = np.ascontiguousarray(X[b]).astype(F8)
        in_maps.append(m)
    if trace:
        _install_profile_hook()
    res = run_bass_kernel_spmd(nc, in_maps, list(range(N_CORES)), trace=trace)
    out = np.empty((B, S, D), dtype=np.float32)
    for b in range(N_CORES):
        r = res.results[b]
        out[b] = (
            np.asarray(r["out"]).astype(np.float32)
            + np.asarray(r["out2"]).astype(np.float32)
        ).T
    return out, res


def _install_profile_hook():
    import antenv

    if "antenv.axon_hooks" not in sys.modules:
        mod = types.ModuleType("antenv.axon_hooks")
        mod._hook = None
        mod.set_axon_ntff_profile_hook = lambda h: setattr(mod, "_hook", h)
        mod.get_axon_ntff_profile_hook = lambda: mod._hook
        sys.modules["antenv.axon_hooks"] = mod
        antenv.axon_hooks = mod
    hooks = sys.modules["antenv.axon_hooks"]
    if hooks.get_axon_ntff_profile_hook() is None:
        from trn_agent_boot.trn_boot import _ntff_profile_via_ctypes

        hooks.set_axon_ntff_profile_hook(
            _ntff_profile_via_ctypes("/opt/axon/libaxon_pjrt.so")
        )
    import concourse.bass_utils as bass_utils

    bass_utils.upload_artifacts = lambda tmpdir: f"local:{tmpdir}"


def kernel(**inputs) -> np.ndarray:
    out, _ = _run(inputs, trace=False)
    return out
